# revision 17
# baseline (speedup 1.0000x reference)
"""Causal multi-head self-attention with RoPE on 8 Trainium2 NeuronCores.

Full-input contract: kernel(**inputs) takes the complete tensors and returns
the complete [B, S, D] output. Core c handles batch c//2 and heads
(c%2)*8 .. (c%2)*8+8; each core emits its partial output projection and the
host sums partner pairs while unsharding (no on-device collective).

v2 design:
  - bf16 matmul inputs throughout (f32 PSUM accumulation), halving DMA/SBUF
    traffic; same PE rate as f32r at these tile sizes.
  - Per-head [E|O] weight-row permutation makes each head's rotated dims 64
    contiguous partitions, so scores are ONE K=64 matmul per (head, k-tile)
    instead of two K=32 matmuls.
  - 2-head-merged [128, 2, 512] PSUM tiles halve ACT/DVE instruction counts.
  - Phase A: QKV projection + RoPE for the whole sequence (DVE-heavy).
    Phase B: per 512-query chunk, attention (ACT/PE) + output projection,
    partials DMA'd straight from PSUM to DRAM.
"""

import numpy as np

import concourse.bass as bass
import concourse.mybir as mybir
import concourse.tile as tile
from concourse import bacc
from concourse.bass_utils import run_bass_kernel_spmd

F32 = mybir.dt.float32
BF16 = mybir.dt.bfloat16
AF = mybir.ActivationFunctionType
ALU = mybir.AluOpType

P = 128          # partitions
SQ = 512         # query-chunk size
DK = 64          # head dim
NH = 8           # heads per core
DLOC = NH * DK   # 512 local out-features for q/k/v
THETA = 10000.0

B, S, D, H = 4, 2048, 1024, 16
N_CORES = 8

NP_BF16 = mybir.dt.np(BF16)


def build_attention_program(reps=1):
    """One SPMD Bass program. Per-core DRAM I/O:
      xt   [D, S]        x[b].T                     bf16
      wqt  [D, DLOC]     wq rows (perm2) transposed  bf16
      wkt  [D, DLOC]     likewise                    bf16
      wvt  [D, DLOC]     likewise                    bf16
      wot  [DLOC, D]     wo cols (perm2) transposed  bf16
      cos4 [P, S]        cos table, 4x stacked [32, S]  f32
      sin4 [P, S]
      outp [D, S]        partial out-projection^T    f32
    """
    KC = D // P          # 8 contraction chunks
    SJ = S // SQ         # 4 query chunks
    STJ = SQ // P        # 4 k-tiles per chunk
    ST = S // P          # 16 k-tiles total
    OC = D // P          # 8 out-feature chunks

    nc = bacc.Bacc("TRN2", target_bir_lowering=False, debug=False, num_devices=1)
    xt = nc.declare_dram_parameter("xt", [D, S], BF16, isOutput=False)
    wqt = nc.declare_dram_parameter("wqt", [D, DLOC], BF16, isOutput=False)
    wkt = nc.declare_dram_parameter("wkt", [D, DLOC], BF16, isOutput=False)
    wvt = nc.declare_dram_parameter("wvt", [D, DLOC], BF16, isOutput=False)
    wot = nc.declare_dram_parameter("wot", [DLOC, D], BF16, isOutput=False)
    cos4 = nc.declare_dram_parameter("cos4", [P, S], F32, isOutput=False)
    sin4 = nc.declare_dram_parameter("sin4", [P, S], F32, isOutput=False)
    outp = nc.declare_dram_parameter("outp", [D, S], F32, isOutput=True)

    from contextlib import ExitStack

    with tile.TileContext(nc) as tc, ExitStack() as ctx:
        ctx.enter_context(nc.allow_low_precision(reason="bf16 inputs, f32 accum"))
        consts = ctx.enter_context(tc.tile_pool(name="consts", bufs=1))
        wload = ctx.enter_context(tc.tile_pool(name="wload", bufs=1))
        xload = ctx.enter_context(tc.tile_pool(name="xload", bufs=2))
        qk_pool = ctx.enter_context(tc.tile_pool(name="qk", bufs=1))
        v_pool = ctx.enter_context(tc.tile_pool(name="vp", bufs=1))
        ot_pool = ctx.enter_context(tc.tile_pool(name="ot", bufs=2))
        tmp_pool = ctx.enter_context(tc.tile_pool(name="tmp", bufs=2))
        pt_pool = ctx.enter_context(tc.tile_pool(name="pt", bufs=6))
        den_pool = ctx.enter_context(tc.tile_pool(name="den", bufs=2))
        ob_pool = ctx.enter_context(tc.tile_pool(name="ob", bufs=2))
        psS = ctx.enter_context(tc.tile_pool(name="psS", bufs=2, space="PSUM"))
        psV = ctx.enter_context(tc.tile_pool(name="psV", bufs=4, space="PSUM"))

        for _rep in range(reps):
            ones_f32 = consts.tile([P, 1], F32, tag="one1")
            nc.vector.memset(ones_f32[:], 1.0)
            # ones64: [1, 64] for K=1 partition-broadcast matmuls
            ones64 = consts.tile([1, DK], BF16, tag="ones64")
            nc.vector.memset(ones64[:], 1.0)

            # --- persistent weight / table loads, spread across DGE queues ---
            wq_sb = wload.tile([P, KC, DLOC], BF16, tag="wq")
            nc.scalar.dma_start(wq_sb[:], wqt.rearrange("(k p) c -> p k c", p=P))
            wk_sb = wload.tile([P, KC, DLOC], BF16, tag="wk")
            nc.gpsimd.dma_start(wk_sb[:], wkt.rearrange("(k p) c -> p k c", p=P))
            wv_sb = wload.tile([P, KC, DLOC], BF16, tag="wv")
            nc.gpsimd.dma_start(wv_sb[:], wvt.rearrange("(k p) c -> p k c", p=P))
            wo_sb = wload.tile([P, DLOC // P, D], BF16, tag="wo")
            nc.sync.dma_start(wo_sb[:], wot.rearrange("(k p) c -> p k c", p=P))
            cos_sb = wload.tile([P, S], F32, tag="cos")
            nc.scalar.dma_start(cos_sb[:], cos4[:])
            sin_sb = wload.tile([P, S], F32, tag="sin")
            nc.sync.dma_start(sin_sb[:], sin4[:])

            # q/k rotated, merged layout: tile m in {0,1} covers heads
            # 4m..4m+3; [:, sub, :] = feature-tile 2m+sub = heads 4m+2sub,
            # 4m+2sub+1, rows per head: [E(32) | O(32)].
            q64 = [
                qk_pool.tile([P, 2, S], BF16, tag=f"q64_{m}", name=f"q64_{m}")
                for m in range(2)
            ]
            k64 = [
                qk_pool.tile([P, 2, S], BF16, tag=f"k64_{m}", name=f"k64_{m}")
                for m in range(2)
            ]
            # v natural [s, dv]: per s-tile, per head: 64 dims + ones col
            v_sb = v_pool.tile([P, ST, NH, DK + 1], BF16, tag="v")
            nc.vector.tensor_copy(
                v_sb[:, :, :, DK : DK + 1],
                ones_f32[:, None, None, :].broadcast_to((P, ST, NH, 1)),
            )

            # ------------ fused per-chunk loop: QKV+RoPE, attention, out-proj ------------
            for j in range(SJ):
                js = slice(j * SQ, (j + 1) * SQ)
                xt_sb = xload.tile([P, KC, SQ], BF16, tag="xt")
                nc.sync.dma_start(
                    xt_sb[:], xt[:, js].rearrange("(k p) s -> p k s", p=P)
                )

                # q/k: merged psum [128, 2, 512]; sub 0 = E-block (evens of
                # heads 4m..4m+3, 32 rows each), sub 1 = O-block (odds).
                for tname, wsb, dst in (("q", wq_sb, q64), ("k", wk_sb, k64)):
                    for m in range(2):
                        ps = psS.tile([P, 2, SQ], F32, tag="sc", name=f"ps_{tname}{m}")
                        for sub in range(2):
                            c0 = (2 * m + sub) * P
                            for kk in range(KC):
                                nc.tensor.matmul(
                                    ps[:, sub, :],
                                    lhsT=(wsb[:, kk, c0 : c0 + P]),
                                    rhs=(xt_sb[:, kk, :]),
                                    start=(kk == 0),
                                    stop=(kk == KC - 1),
                                )
                        # RoPE: yE = cos*E - sin*O ; yO = sin*E + cos*O.
                        # E/O separation lives in the free dim (sub), so all
                        # tensor_tensor inputs share start partitions.
                        cj = cos_sb[:, js]
                        sj_ = sin_sb[:, js]
                        t1 = tmp_pool.tile([P, 2, SQ], BF16, tag="t1")
                        nc.vector.tensor_tensor(
                            t1[:], cj[:, None, :].broadcast_to((P, 2, SQ)), ps[:], ALU.mult
                        )
                        t2 = tmp_pool.tile([P, 2, SQ], BF16, tag="t2")
                        nc.vector.tensor_tensor(
                            t2[:], sj_[:, None, :].broadcast_to((P, 2, SQ)), ps[:], ALU.mult
                        )
                        dv = dst[m]
                        for a in range(4):  # head 4m+a; rows a*32..a*32+32
                            ra = a * 32
                            rd = (a % 2) * 64
                            nc.vector.tensor_tensor(
                                dv[rd : rd + 32, a // 2, js],
                                t1[ra : ra + 32, 0, :],
                                t2[ra : ra + 32, 1, :],
                                ALU.subtract,
                            )
                            nc.vector.tensor_tensor(
                                dv[rd + 32 : rd + 64, a // 2, js],
                                t2[ra : ra + 32, 0, :],
                                t1[ra : ra + 32, 1, :],
                                ALU.add,
                            )

                # v: psum [s 128, dv 512] per s-tile; merged 2 s-tiles/psum;
                # psum->SBUF bf16 copy on the scalar engine (idle here).
                for m in range(2):
                    ps = psS.tile([P, 2, SQ], F32, tag="sc", name=f"ps_v{m}")
                    for sub in range(2):
                        st = j * STJ + 2 * m + sub
                        s0 = (2 * m + sub) * P
                        for kk in range(KC):
                            nc.tensor.matmul(
                                ps[:, sub, :],
                                lhsT=(xt_sb[:, kk, s0 : s0 + P]),
                                rhs=(wv_sb[:, kk, :]),
                                start=(kk == 0),
                                stop=(kk == KC - 1),
                            )
                    nc.scalar.copy(
                        v_sb[:, j * STJ + 2 * m : j * STJ + 2 * m + 2, :, 0:DK],
                        ps.rearrange("p a (h d) -> p a h d", h=NH),
                    )

                # ---- attention for chunk j ----
                ntile = (j + 1) * STJ
                ot_sb = [
                    ot_pool.tile([P, SQ], BF16, tag=f"ot{i}", name=f"ot{i}")
                    for i in range(4)
                ]
                for hg in range(2):  # heads 4*hg .. 4*hg+3
                    m = hg
                    opvs = [
                        psV.tile([DK + 1, SQ], F32, tag="pv", name=f"pv{h}")
                        for h in range(4)
                    ]
                    pts = {}
                    for t in range(ntile):
                        ts_ = slice(t * P, (t + 1) * P)
                        # scores: pair = (head 4m+2sub, 4m+2sub+1)
                        for sub in range(2):
                            ssc = psS.tile([P, 2, SQ], F32, tag="sc", name="ssc")
                            for hh in range(2):
                                r = hh * 64
                                nc.tensor.matmul(
                                    ssc[:, hh, :],
                                    lhsT=(k64[m][r : r + DK, sub, ts_]),
                                    rhs=(q64[m][r : r + DK, sub, js]),
                                    start=True,
                                    stop=True,
                                )
                            pt = pt_pool.tile([P, 2, SQ], BF16, tag="pt")
                            nc.scalar.activation(pt[:], ssc[:], AF.Exp, scale=0.125)
                            if t >= ntile - STJ:
                                nc.gpsimd.affine_select(
                                    out=pt[:],
                                    in_=pt[:],
                                    compare_op=ALU.is_ge,
                                    fill=0.0,
                                    base=j * SQ - t * P,
                                    pattern=[[0, 2], [1, SQ]],
                                    channel_multiplier=-1,
                                )
                            pts[sub] = pt
                        for h in range(4):
                            head = 4 * hg + h
                            nc.tensor.matmul(
                                opvs[h][:],
                                lhsT=(v_sb[:, t, head, :]),
                                rhs=(pts[h // 2][:, h % 2, :]),
                                start=(t == 0),
                                stop=(t == ntile - 1),
                            )
                    # normalize: 1/denom broadcast across partitions 0..63 via
                    # K=1 matmuls; heads separated along the free dim so the
                    # scale inputs stay partition-aligned with opv rows 0..63.
                    den1 = den_pool.tile([1, 4, SQ], BF16, tag="den1")
                    for h in range(4):
                        nc.vector.reciprocal(den1[0:1, h, :], opvs[h][DK : DK + 1, :])
                    denbs = []
                    for pr in range(2):  # heads 2pr, 2pr+1
                        psb = psS.tile([P, 2, SQ], F32, tag="sc", name="psb")
                        for hh in range(2):
                            nc.tensor.matmul(
                                psb[0:DK, hh, :],
                                lhsT=(ones64[:]),
                                rhs=(den1[0:1, 2 * pr + hh, :]),
                                start=True,
                                stop=True,
                            )
                        denb = den_pool.tile([DK, 2, SQ], BF16, tag=f"denb{pr}")
                        nc.vector.tensor_copy(denb[:], psb[0:DK, :, :])
                        denbs.append(denb)
                    for h in range(4):
                        head = 4 * hg + h
                        nc.vector.tensor_tensor(
                            ot_sb[head // 2][(head % 2) * DK : (head % 2 + 1) * DK, :],
                            opvs[h][0:DK, :],
                            denbs[h // 2][0:DK, h % 2, :],
                            ALU.mult,
                        )
                # output projection for chunk j; DMA partials straight from PSUM
                for dc in range(OC):
                    ps = psV.tile([P, SQ], F32, tag="pv", name="psout")
                    for ic in range(4):
                        nc.tensor.matmul(
                            ps[:],
                            lhsT=(wo_sb[:, ic, dc * P : (dc + 1) * P]),
                            rhs=(ot_sb[ic][:]),
                            start=(ic == 0),
                            stop=(ic == 3),
                        )
                    ob = ob_pool.tile([P, SQ], F32, tag="ob")
                    if dc % 2 == 0:
                        nc.vector.tensor_copy(ob[:], ps[:])
                    else:
                        nc.scalar.copy(ob[:], ps[:])
                    nc.sync.dma_start(outp[dc * P : (dc + 1) * P, js], ob[:])

    nc.finalize()
    return nc


def make_perms():
    """perm (q/k): per 4-head group: E-block (evens of the 4 heads) then
    O-block (odds). perm2 (v/wo): per head, [even dims | odd dims].
    Both local to a core's 512 rows."""
    perm = []
    for grp in range(2):
        for par in range(2):
            for h in range(4 * grp, 4 * grp + 4):
                for i in range(32):
                    perm.append(h * DK + 2 * i + par)
    perm2 = []
    for h in range(NH):
        for par in range(2):
            for i in range(32):
                perm2.append(h * DK + 2 * i + par)
    return np.array(perm), np.array(perm2)


def make_tables(token_positions):
    pos = np.asarray(token_positions).astype(np.float32)
    inv_freq = (1.0 / (THETA ** (np.arange(0, DK, 2, dtype=np.float32) / DK))).astype(
        np.float32
    )
    freqs = pos[:, None] * inv_freq[None, :]  # [S, 32]
    cosT = np.cos(freqs).T.astype(np.float32)  # [32, S]
    sinT = np.sin(freqs).T.astype(np.float32)
    return (
        np.ascontiguousarray(np.tile(cosT, (4, 1))),
        np.ascontiguousarray(np.tile(sinT, (4, 1))),
    )


def shard_inputs(x, token_positions, wq, wk, wv, wo):
    """Build the 8 per-core input maps."""
    perm, perm2 = make_perms()
    cos4, sin4 = make_tables(token_positions)
    in_maps = []
    for c in range(N_CORES):
        b, hg = c // 2, c % 2
        gperm = perm + hg * DLOC
        gperm2 = perm2 + hg * DLOC
        in_maps.append(
            {
                "xt": np.ascontiguousarray(x[b].T).astype(NP_BF16),
                "wqt": np.ascontiguousarray(wq[gperm, :].T.astype(NP_BF16)),
                "wkt": np.ascontiguousarray(wk[gperm, :].T.astype(NP_BF16)),
                "wvt": np.ascontiguousarray(wv[gperm2, :].T.astype(NP_BF16)),
                "wot": np.ascontiguousarray(wo[:, gperm2].T.astype(NP_BF16)),
                "cos4": cos4,
                "sin4": sin4,
            }
        )
    return in_maps


_NC_CACHE = {}


def build_program(reps=1):
    return build_attention_program(reps=reps)


def assemble_output(res):
    out = np.empty((B, S, D), dtype=np.float32)
    for b in range(B):
        pT = res.results[2 * b]["outp"] + res.results[2 * b + 1]["outp"]  # [D, S]
        out[b] = pT.T
    return out


def kernel(x, token_positions, wq, wk, wv, wo, trace=False):
    x = np.asarray(x, dtype=np.float32)
    wq = np.asarray(wq, dtype=np.float32)
    wk = np.asarray(wk, dtype=np.float32)
    wv = np.asarray(wv, dtype=np.float32)
    wo = np.asarray(wo, dtype=np.float32)

    key = "full"
    if key not in _NC_CACHE:
        _NC_CACHE[key] = build_program()
    nc = _NC_CACHE[key]

    in_maps = shard_inputs(x, token_positions, wq, wk, wv, wo)
    res = run_bass_kernel_spmd(nc, in_maps, list(range(N_CORES)), trace=trace)
    out = assemble_output(res)
    if trace:
        return out, res
    return out


# revision 19
# speedup vs baseline: 1.2340x; 1.2340x over previous
"""Causal multi-head self-attention with RoPE on 8 Trainium2 NeuronCores.

Full-input contract: kernel(**inputs) takes the complete tensors and returns
the complete [B, S, D] output. Core c handles batch c//2 and heads
(c%2)*8 .. (c%2)*8+8; each core emits its partial output projection and the
host sums partner pairs while unsharding (no on-device collective).

v2 design:
  - bf16 matmul inputs throughout (f32 PSUM accumulation), halving DMA/SBUF
    traffic; same PE rate as f32r at these tile sizes.
  - Per-head [E|O] weight-row permutation makes each head's rotated dims 64
    contiguous partitions, so scores are ONE K=64 matmul per (head, k-tile)
    instead of two K=32 matmuls.
  - 2-head-merged [128, 2, 512] PSUM tiles halve ACT/DVE instruction counts.
  - Phase A: QKV projection + RoPE for the whole sequence (DVE-heavy).
    Phase B: per 512-query chunk, attention (ACT/PE) + output projection,
    partials DMA'd straight from PSUM to DRAM.
"""

import numpy as np

import concourse.bass as bass
import concourse.mybir as mybir
import concourse.tile as tile
from concourse import bacc
from concourse.bass_utils import run_bass_kernel_spmd

F32 = mybir.dt.float32
BF16 = mybir.dt.bfloat16
AF = mybir.ActivationFunctionType
ALU = mybir.AluOpType

P = 128          # partitions
SQ = 512         # query-chunk size
DK = 64          # head dim
NH = 8           # heads per core
DLOC = NH * DK   # 512 local out-features for q/k/v
THETA = 10000.0

B, S, D, H = 4, 2048, 1024, 16
N_CORES = 8

NP_BF16 = mybir.dt.np(BF16)


def build_attention_program(reps=1):
    """One SPMD Bass program. Per-core DRAM I/O:
      xt   [D, S]        x[b].T                     bf16
      wqt  [D, DLOC]     wq rows (perm2) transposed  bf16
      wkt  [D, DLOC]     likewise                    bf16
      wvt  [D, DLOC]     likewise                    bf16
      wot  [DLOC, D]     wo cols (perm2) transposed  bf16
      cos4 [P, S]        cos table, 4x stacked [32, S]  f32
      sin4 [P, S]
      outp [D, S]        partial out-projection^T    f32
    """
    KC = D // P          # 8 contraction chunks
    SJ = S // SQ         # 4 query chunks
    STJ = SQ // P        # 4 k-tiles per chunk
    ST = S // P          # 16 k-tiles total
    OC = D // P          # 8 out-feature chunks

    nc = bacc.Bacc("TRN2", target_bir_lowering=False, debug=False, num_devices=1)
    xt = nc.declare_dram_parameter("xt", [D, S], BF16, isOutput=False)
    wqt = nc.declare_dram_parameter("wqt", [D, DLOC], BF16, isOutput=False)
    wkt = nc.declare_dram_parameter("wkt", [D, DLOC], BF16, isOutput=False)
    wvt = nc.declare_dram_parameter("wvt", [D, DLOC], BF16, isOutput=False)
    wot = nc.declare_dram_parameter("wot", [DLOC, D], BF16, isOutput=False)
    cos4 = nc.declare_dram_parameter("cos4", [P, S], F32, isOutput=False)
    sin4 = nc.declare_dram_parameter("sin4", [P, S], F32, isOutput=False)
    outp = nc.declare_dram_parameter("outp", [D, S], F32, isOutput=True)

    from contextlib import ExitStack

    with tile.TileContext(nc) as tc, ExitStack() as ctx:
        ctx.enter_context(nc.allow_low_precision(reason="bf16 inputs, f32 accum"))
        consts = ctx.enter_context(tc.tile_pool(name="consts", bufs=1))
        wload = ctx.enter_context(tc.tile_pool(name="wload", bufs=1))
        xload = ctx.enter_context(tc.tile_pool(name="xload", bufs=2))
        qk_pool = ctx.enter_context(tc.tile_pool(name="qk", bufs=1))
        v_pool = ctx.enter_context(tc.tile_pool(name="vp", bufs=1))
        ot_pool = ctx.enter_context(tc.tile_pool(name="ot", bufs=2))
        tmp_pool = ctx.enter_context(tc.tile_pool(name="tmp", bufs=2))
        pt_pool = ctx.enter_context(tc.tile_pool(name="pt", bufs=6))
        den_pool = ctx.enter_context(tc.tile_pool(name="den", bufs=2))
        ob_pool = ctx.enter_context(tc.tile_pool(name="ob", bufs=2))
        psS = ctx.enter_context(tc.tile_pool(name="psS", bufs=2, space="PSUM"))
        psV = ctx.enter_context(tc.tile_pool(name="psV", bufs=4, space="PSUM"))

        for _rep in range(reps):
            ones_f32 = consts.tile([P, 1], F32, tag="one1")
            nc.vector.memset(ones_f32[:], 1.0)
            # ones64: [1, 64] for K=1 partition-broadcast matmuls
            ones64 = consts.tile([1, DK], BF16, tag="ones64")
            nc.vector.memset(ones64[:], 1.0)

            # --- persistent weight / table loads, spread across DGE queues ---
            wq_sb = wload.tile([P, KC, DLOC], BF16, tag="wq")
            nc.scalar.dma_start(wq_sb[:], wqt.rearrange("(k p) c -> p k c", p=P))
            wk_sb = wload.tile([P, KC, DLOC], BF16, tag="wk")
            nc.gpsimd.dma_start(wk_sb[:], wkt.rearrange("(k p) c -> p k c", p=P))
            wv_sb = wload.tile([P, KC, DLOC], BF16, tag="wv")
            nc.gpsimd.dma_start(wv_sb[:], wvt.rearrange("(k p) c -> p k c", p=P))
            wo_sb = wload.tile([P, DLOC // P, D], BF16, tag="wo")
            nc.sync.dma_start(wo_sb[:], wot.rearrange("(k p) c -> p k c", p=P))
            cos_sb = wload.tile([P, S], F32, tag="cos")
            nc.scalar.dma_start(cos_sb[:], cos4[:])
            sin_sb = wload.tile([P, S], F32, tag="sin")
            nc.sync.dma_start(sin_sb[:], sin4[:])

            # q/k rotated, merged layout: tile m in {0,1} covers heads
            # 4m..4m+3; [:, sub, :] = feature-tile 2m+sub = heads 4m+2sub,
            # 4m+2sub+1, rows per head: [E(32) | O(32)].
            q64 = [
                qk_pool.tile([P, 2, S], BF16, tag=f"q64_{m}", name=f"q64_{m}")
                for m in range(2)
            ]
            k64 = [
                qk_pool.tile([P, 2, S], BF16, tag=f"k64_{m}", name=f"k64_{m}")
                for m in range(2)
            ]
            # v natural [s, dv]: per s-tile, per head: 64 dims + ones col
            v_sb = v_pool.tile([P, ST, NH, DK + 1], BF16, tag="v")
            nc.vector.tensor_copy(
                v_sb[:, :, :, DK : DK + 1],
                ones_f32[:, None, None, :].broadcast_to((P, ST, NH, 1)),
            )

            # ---------------- Phase A: QKV projections + RoPE ----------------
            for j in range(SJ):
                js = slice(j * SQ, (j + 1) * SQ)
                xt_sb = xload.tile([P, KC, SQ], BF16, tag="xt")
                nc.sync.dma_start(
                    xt_sb[:], xt[:, js].rearrange("(k p) s -> p k s", p=P)
                )

                # q/k: merged psum [128, 2, 512]; sub 0 = E-block (evens of
                # heads 4m..4m+3, 32 rows each), sub 1 = O-block (odds).
                for tname, wsb, dst in (("q", wq_sb, q64), ("k", wk_sb, k64)):
                    for m in range(2):
                        ps = psS.tile([P, 2, SQ], F32, tag="sc", name=f"ps_{tname}{m}")
                        for sub in range(2):
                            c0 = (2 * m + sub) * P
                            for kk in range(KC):
                                nc.tensor.matmul(
                                    ps[:, sub, :],
                                    lhsT=(wsb[:, kk, c0 : c0 + P]),
                                    rhs=(xt_sb[:, kk, :]),
                                    start=(kk == 0),
                                    stop=(kk == KC - 1),
                                )
                        # RoPE: yE = cos*E - sin*O ; yO = sin*E + cos*O.
                        # E/O separation lives in the free dim (sub), so all
                        # tensor_tensor inputs share start partitions.
                        cj = cos_sb[:, js]
                        sj_ = sin_sb[:, js]
                        t1 = tmp_pool.tile([P, 2, SQ], BF16, tag="t1")
                        nc.vector.tensor_tensor(
                            t1[:], cj[:, None, :].broadcast_to((P, 2, SQ)), ps[:], ALU.mult
                        )
                        t2 = tmp_pool.tile([P, 2, SQ], BF16, tag="t2")
                        nc.vector.tensor_tensor(
                            t2[:], sj_[:, None, :].broadcast_to((P, 2, SQ)), ps[:], ALU.mult
                        )
                        dv = dst[m]
                        for a in range(4):  # head 4m+a; rows a*32..a*32+32
                            ra = a * 32
                            rd = (a % 2) * 64
                            nc.vector.tensor_tensor(
                                dv[rd : rd + 32, a // 2, js],
                                t1[ra : ra + 32, 0, :],
                                t2[ra : ra + 32, 1, :],
                                ALU.subtract,
                            )
                            nc.vector.tensor_tensor(
                                dv[rd + 32 : rd + 64, a // 2, js],
                                t2[ra : ra + 32, 0, :],
                                t1[ra : ra + 32, 1, :],
                                ALU.add,
                            )

                # v: psum [s 128, dv 512] per s-tile; merged 2 s-tiles/psum
                for m in range(2):
                    ps = psS.tile([P, 2, SQ], F32, tag="sc", name=f"ps_v{m}")
                    for sub in range(2):
                        st = j * STJ + 2 * m + sub
                        s0 = (2 * m + sub) * P
                        for kk in range(KC):
                            nc.tensor.matmul(
                                ps[:, sub, :],
                                lhsT=(xt_sb[:, kk, s0 : s0 + P]),
                                rhs=(wv_sb[:, kk, :]),
                                start=(kk == 0),
                                stop=(kk == KC - 1),
                            )
                    nc.vector.tensor_copy(
                        v_sb[:, j * STJ + 2 * m : j * STJ + 2 * m + 2, :, 0:DK],
                        ps.rearrange("p a (h d) -> p a h d", h=NH),
                    )

            # ---------------- Phase B: attention + output projection ----------------
            for j in range(SJ):
                js = slice(j * SQ, (j + 1) * SQ)
                ntile = (j + 1) * STJ
                ot_sb = [
                    ot_pool.tile([P, SQ], BF16, tag=f"ot{i}", name=f"ot{i}")
                    for i in range(4)
                ]
                for hg in range(2):  # heads 4*hg .. 4*hg+3
                    m = hg
                    opvs = [
                        psV.tile([DK + 1, SQ], F32, tag="pv", name=f"pv{h}")
                        for h in range(4)
                    ]
                    pts = {}
                    for t in range(ntile):
                        ts_ = slice(t * P, (t + 1) * P)
                        # scores: pair = (head 4m+2sub, 4m+2sub+1)
                        for sub in range(2):
                            ssc = psS.tile([P, 2, SQ], F32, tag="sc", name="ssc")
                            for hh in range(2):
                                r = hh * 64
                                nc.tensor.matmul(
                                    ssc[:, hh, :],
                                    lhsT=(k64[m][r : r + DK, sub, ts_]),
                                    rhs=(q64[m][r : r + DK, sub, js]),
                                    start=True,
                                    stop=True,
                                )
                            pt = pt_pool.tile([P, 2, SQ], BF16, tag="pt")
                            nc.scalar.activation(pt[:], ssc[:], AF.Exp, scale=0.125)
                            if t >= ntile - STJ:
                                nc.gpsimd.affine_select(
                                    out=pt[:],
                                    in_=pt[:],
                                    compare_op=ALU.is_ge,
                                    fill=0.0,
                                    base=j * SQ - t * P,
                                    pattern=[[0, 2], [1, SQ]],
                                    channel_multiplier=-1,
                                )
                            pts[sub] = pt
                        for h in range(4):
                            head = 4 * hg + h
                            nc.tensor.matmul(
                                opvs[h][:],
                                lhsT=(v_sb[:, t, head, :]),
                                rhs=(pts[h // 2][:, h % 2, :]),
                                start=(t == 0),
                                stop=(t == ntile - 1),
                            )
                    # normalize: 1/denom broadcast across partitions 0..63 via
                    # K=1 matmuls; heads separated along the free dim so the
                    # scale inputs stay partition-aligned with opv rows 0..63.
                    den1 = den_pool.tile([1, 4, SQ], BF16, tag="den1")
                    for h in range(4):
                        nc.vector.reciprocal(den1[0:1, h, :], opvs[h][DK : DK + 1, :])
                    denbs = []
                    for pr in range(2):  # heads 2pr, 2pr+1
                        psb = psS.tile([P, 2, SQ], F32, tag="sc", name="psb")
                        for hh in range(2):
                            nc.tensor.matmul(
                                psb[0:DK, hh, :],
                                lhsT=(ones64[:]),
                                rhs=(den1[0:1, 2 * pr + hh, :]),
                                start=True,
                                stop=True,
                            )
                        denb = den_pool.tile([DK, 2, SQ], BF16, tag=f"denb{pr}")
                        nc.vector.tensor_copy(denb[:], psb[0:DK, :, :])
                        denbs.append(denb)
                    for h in range(4):
                        head = 4 * hg + h
                        nc.vector.tensor_tensor(
                            ot_sb[head // 2][(head % 2) * DK : (head % 2 + 1) * DK, :],
                            opvs[h][0:DK, :],
                            denbs[h // 2][0:DK, h % 2, :],
                            ALU.mult,
                        )
                # output projection for chunk j; DMA partials straight from PSUM
                for dc in range(OC):
                    ps = psV.tile([P, SQ], F32, tag="pv", name="psout")
                    for ic in range(4):
                        nc.tensor.matmul(
                            ps[:],
                            lhsT=(wo_sb[:, ic, dc * P : (dc + 1) * P]),
                            rhs=(ot_sb[ic][:]),
                            start=(ic == 0),
                            stop=(ic == 3),
                        )
                    ob = ob_pool.tile([P, SQ], F32, tag="ob")
                    nc.vector.tensor_copy(ob[:], ps[:])
                    nc.sync.dma_start(outp[dc * P : (dc + 1) * P, js], ob[:])

    nc.finalize()
    return nc


def make_perms():
    """perm (q/k): per 4-head group: E-block (evens of the 4 heads) then
    O-block (odds). perm2 (v/wo): per head, [even dims | odd dims].
    Both local to a core's 512 rows."""
    perm = []
    for grp in range(2):
        for par in range(2):
            for h in range(4 * grp, 4 * grp + 4):
                for i in range(32):
                    perm.append(h * DK + 2 * i + par)
    perm2 = []
    for h in range(NH):
        for par in range(2):
            for i in range(32):
                perm2.append(h * DK + 2 * i + par)
    return np.array(perm), np.array(perm2)


def make_tables(token_positions):
    pos = np.asarray(token_positions).astype(np.float32)
    inv_freq = (1.0 / (THETA ** (np.arange(0, DK, 2, dtype=np.float32) / DK))).astype(
        np.float32
    )
    freqs = pos[:, None] * inv_freq[None, :]  # [S, 32]
    cosT = np.cos(freqs).T.astype(np.float32)  # [32, S]
    sinT = np.sin(freqs).T.astype(np.float32)
    return (
        np.ascontiguousarray(np.tile(cosT, (4, 1))),
        np.ascontiguousarray(np.tile(sinT, (4, 1))),
    )


def shard_inputs(x, token_positions, wq, wk, wv, wo):
    """Build the 8 per-core input maps."""
    perm, perm2 = make_perms()
    cos4, sin4 = make_tables(token_positions)
    in_maps = []
    for c in range(N_CORES):
        b, hg = c // 2, c % 2
        gperm = perm + hg * DLOC
        gperm2 = perm2 + hg * DLOC
        in_maps.append(
            {
                "xt": np.ascontiguousarray(x[b].T).astype(NP_BF16),
                "wqt": np.ascontiguousarray(wq[gperm, :].T.astype(NP_BF16)),
                "wkt": np.ascontiguousarray(wk[gperm, :].T.astype(NP_BF16)),
                "wvt": np.ascontiguousarray(wv[gperm2, :].T.astype(NP_BF16)),
                "wot": np.ascontiguousarray(wo[:, gperm2].T.astype(NP_BF16)),
                "cos4": cos4,
                "sin4": sin4,
            }
        )
    return in_maps


_NC_CACHE = {}


def build_program(reps=1):
    return build_attention_program(reps=reps)


def assemble_output(res):
    out = np.empty((B, S, D), dtype=np.float32)
    for b in range(B):
        pT = res.results[2 * b]["outp"] + res.results[2 * b + 1]["outp"]  # [D, S]
        out[b] = pT.T
    return out


def kernel(x, token_positions, wq, wk, wv, wo, trace=False):
    x = np.asarray(x, dtype=np.float32)
    wq = np.asarray(wq, dtype=np.float32)
    wk = np.asarray(wk, dtype=np.float32)
    wv = np.asarray(wv, dtype=np.float32)
    wo = np.asarray(wo, dtype=np.float32)

    key = "full"
    if key not in _NC_CACHE:
        _NC_CACHE[key] = build_program()
    nc = _NC_CACHE[key]

    in_maps = shard_inputs(x, token_positions, wq, wk, wv, wo)
    res = run_bass_kernel_spmd(nc, in_maps, list(range(N_CORES)), trace=trace)
    out = assemble_output(res)
    if trace:
        return out, res
    return out
